# revision 19
# baseline (speedup 1.0000x reference)
"""Trainium2 Bass kernel for the per-channel CDF-flow MLP (position-sharded).

Math (per channel c, elementwise over N positions):
    u0 = W0 x + b0          v0 = u0 + T0*tanh(u0)     (W* = softplus(m*), T* = tanh(f*))
    u1 = W1 v0 + b1         v1 = u1 + T1*tanh(u1)
    u2 = W2 v1 + b2         v2 = u2 + T2*tanh(u2)
    out = W3 v2 + b3

Sharding: positions (65536) split across 8 cores, 8192 each; every core
holds all 256 channels. The bottleneck engine is ACT (9 tanh per
channel-position); ACT cost is free-dim-proportional and partition-count-
independent, so channels are packed 42-per-tile (126/128 partitions
useful vs 96 in a 32-channel layout):
  - 6 "patterns" of 42 channels -> [126, F] tiles, rows j-major (42j + c)
  - 4-channel tail repacked as 32 (channel, pos-block) units -> [96, F]

Data staging (all resident in SBUF, no per-unit DMA; DMA cost here is
~1us per descriptor, so descriptors are kept at full-row 16-32 KB):
  xE[P] [126, 8192] bf16  x replicated 3x j-major straight from HBM in ONE
                          SWDGE cast DMA per pattern (32 KB f32 source
                          descriptors, 126 per transfer)
  osb[g][126, 4096] f32   output batches; 4 plain HWDGE out-DMAs per repeat
  reloads for the next repeat are emitted right after each tile's last
  reader (iterations 40..46), so they overlap compute across the For_i
  boundary.

Per unit (pattern x 1024-col chunk):
  t0   = tanh(W0*xE + c0)                  (ACT, per-partition scale+bias)
  y1   = A1t.T @ t0 + A1x-cols.T @ x       (PE; A1t = W1 diag(T0), A1x = W1@W0)
  t1   = tanh(y1 + c1)                     (ACT)
  z2   = T1*t1 + y1                        (DVE scalar_tensor_tensor)
  y2   = W2-blockdiag.T @ z2               (PE)
  t2   = tanh(y2 + c2)                     (ACT)
  pack += A3z.T @ z2 + A3t.T @ t2          (PE; 3 patterns -> 126 rows)
  osb[:, t-slice] = pack + c3              (DVE; [126, 2048] out batches)
with c0 = b0, c1 = W1 b0 + b1, c2 = W2 c1 + b2, c3 = W3 c2 + b3.

Software pipeline (stages phased so ACT streams back-to-back):
iteration i emits  B(i+3) t0, C(i+2) y1, D(i+2) t1, E(i+1) z2,
                   F(i+1) y2, G(i+1) t2, H(i) pack.
PSUM: y1 bufs=2 (4 banks) + y2 bufs=1 (2) + pack bufs=1 (2) = 8 banks.
Only the x -> tanh / x -> A1x paths see bf16 (~1e-3 final rel err);
all other matmul operands are f32r.
"""

import os
from contextlib import ExitStack, nullcontext

import ml_dtypes
import numpy as np

import concourse.bacc as bacc
import concourse.bass as bass
import concourse.tile as tile
from concourse import mybir
from concourse.bass_utils import run_bass_kernel_spmd

F32 = mybir.dt.float32
F32R = mybir.dt.float32r
BF16 = mybir.dt.bfloat16

CH = 256
NPOS = 65536
NCORES = 8
POSC = NPOS // NCORES       # 8192 positions per core
CHP = 42                    # channels per pattern tile
NPAT = 6                    # full patterns (252 channels)
R = 3 * CHP                 # 126 rows per pattern tile
TCH = 4                     # tail channels (252..255)
TB = 8                      # tail position blocks of F
TU = TCH * TB               # 32 tail units
TR = 3 * TU                 # 96 tail rows
F = 1024                    # unit free-dim chunk (PSUM tile = 2 banks f32)
MMN = 512                   # matmul free-dim slice (one PSUM bank)
NCHUNK = POSC // F          # 8
NU = NCHUNK * NPAT          # 48 pattern units; unit NU is the tail
HF = POSC                  # xE tiles are full-width (one reload per repeat)
OF = POSC // 2              # osb out tiles: [126, 4096] f32, 2 halves/repeat

LAST_RESULTS = None         # test.py introspects this


def _softplus(x):
    return np.logaddexp(0.0, x.astype(np.float64))


def _host_params(m0, m1, m2, m3, b0, b1, b2, b3, f0, f1, f2):
    """Fold weights/biases/gates into the device parameterization (float64)."""
    W0 = _softplus(m0)[:, :, 0]
    W1 = _softplus(m1)
    W2 = _softplus(m2)
    W3 = _softplus(m3)[:, 0, :]
    b0_ = b0.astype(np.float64)[:, :, 0]
    b1_ = b1.astype(np.float64)[:, :, 0]
    b2_ = b2.astype(np.float64)[:, :, 0]
    b3_ = b3.astype(np.float64)[:, 0, 0]
    T0 = np.tanh(f0.astype(np.float64))[:, :, 0]
    T1 = np.tanh(f1.astype(np.float64))[:, :, 0]
    T2 = np.tanh(f2.astype(np.float64))[:, :, 0]
    c0 = b0_
    c1 = np.einsum("cjk,ck->cj", W1, b0_) + b1_
    c2 = np.einsum("cjk,ck->cj", W2, c1) + b2_
    c3 = np.einsum("ck,ck->c", W3, c2) + b3_
    A1x = np.einsum("cjk,ck->cj", W1, W0)      # W1 @ W0
    A1t = W1 * T0[:, None, :]                  # W1 diag(T0)
    A3z = np.einsum("cm,cmk->ck", W3, W2)      # W3 @ W2
    A3t = W3 * T2                              # W3 diag(T2)
    return dict(A1x=A1x, A1t=A1t, W2=W2, A3z=A3z, A3t=A3t, W0=W0,
                c0=c0, c1=c1, c2=c2, c3=c3, T1=T1)


def _device_arrays(p):
    """Shared (core-independent) device arrays from host params `p`."""
    f32 = np.float32
    bf16 = ml_dtypes.bfloat16
    arrs = {}
    c = np.arange(CHP)
    for P in range(NPAT):
        sl = slice(CHP * P, CHP * P + CHP)
        A1x, A1t, W2 = p["A1x"][sl], p["A1t"][sl], p["W2"][sl]
        A3z, A3t = p["A3z"][sl], p["A3t"][sl]
        l1t = np.zeros((R, R), f32)
        l2 = np.zeros((R, R), f32)
        d1 = np.zeros((CHP, R), f32)
        g = P % 3
        l3z = np.zeros((R, R), f32)
        l3t = np.zeros((R, R), f32)
        for j in range(3):
            d1[c, CHP * j + c] = A1x[:, j]
            for k in range(3):
                l1t[CHP * k + c, CHP * j + c] = A1t[:, j, k]
                l2[CHP * k + c, CHP * j + c] = W2[:, j, k]
        for k in range(3):
            l3z[CHP * k + c, CHP * g + c] = A3z[:, k]
            l3t[CHP * k + c, CHP * g + c] = A3t[:, k]
        vec = lambda t: np.concatenate(
            [t[:, j] for j in range(3)]).astype(f32).reshape(R, 1)
        arrs[f"l1t{P}"] = l1t
        arrs[f"d1_{P}"] = d1.astype(bf16)
        arrs[f"l2_{P}"] = l2
        arrs[f"l3z{P}"] = l3z
        arrs[f"l3t{P}"] = l3t
        arrs[f"W0v{P}"] = vec(p["W0"][sl])
        arrs[f"c0v{P}"] = vec(p["c0"][sl])
        arrs[f"c1v{P}"] = vec(p["c1"][sl])
        arrs[f"c2v{P}"] = vec(p["c2"][sl])
        arrs[f"T1v{P}"] = vec(p["T1"][sl])
    arrs["c3vA"] = p["c3"][0:126].astype(f32).reshape(126, 1)
    arrs["c3vB"] = p["c3"][126:252].astype(f32).reshape(126, 1)
    # tail: channels 252..255, unit u = 8c + b, rows 32j + u
    u = np.arange(TU)
    chu = 252 + u // TB
    l1tT = np.zeros((TR, TR), f32)
    l2T = np.zeros((TR, TR), f32)
    d1T = np.zeros((TU, TR), f32)
    l3zT = np.zeros((TR, TU), f32)
    l3tT = np.zeros((TR, TU), f32)
    for j in range(3):
        d1T[u, TU * j + u] = p["A1x"][chu, j]
        for k in range(3):
            l1tT[TU * k + u, TU * j + u] = p["A1t"][chu, j, k]
            l2T[TU * k + u, TU * j + u] = p["W2"][chu, j, k]
    for k in range(3):
        l3zT[TU * k + u, u] = p["A3z"][chu, k]
        l3tT[TU * k + u, u] = p["A3t"][chu, k]
    vecT = lambda t: np.concatenate(
        [t[chu, j] for j in range(3)]).astype(f32).reshape(TR, 1)
    arrs.update(l1tT=l1tT, d1T=d1T.astype(bf16), l2T=l2T, l3zT=l3zT,
                l3tT=l3tT,
                W0vT=vecT(p["W0"]), c0vT=vecT(p["c0"]), c1vT=vecT(p["c1"]),
                c2vT=vecT(p["c2"]), T1vT=vecT(p["T1"]),
                c3vT=p["c3"][chu].astype(f32).reshape(TU, 1))
    return arrs


# (name, shape, dtype) of every device parameter
def _param_specs():
    specs = []
    for P in range(NPAT):
        specs += [(f"l1t{P}", [R, R], F32R), (f"d1_{P}", [CHP, R], BF16),
                  (f"l2_{P}", [R, R], F32R), (f"l3z{P}", [R, R], F32R),
                  (f"l3t{P}", [R, R], F32R)]
        specs += [(f"{v}{P}", [R, 1], F32)
                  for v in ("W0v", "c0v", "c1v", "c2v", "T1v")]
    specs += [("c3vA", [126, 1], F32), ("c3vB", [126, 1], F32)]
    specs += [("l1tT", [TR, TR], F32R), ("d1T", [TU, TR], BF16),
              ("l2T", [TR, TR], F32R), ("l3zT", [TR, TU], F32R),
              ("l3tT", [TR, TU], F32R)]
    specs += [(f"{v}T", [TR, 1], F32)
              for v in ("W0v", "c0v", "c1v", "c2v", "T1v")]
    specs += [("c3vT", [TU, 1], F32)]
    return specs


def build_nc(repeat=1, variant="full"):
    """Build the per-core Bass program (SPMD: same program + weights,
    per-core x/o). variant: full | compute_only | dma_only"""
    do_dma = variant != "compute_only"
    do_compute = variant != "dma_only"
    nc = bacc.Bacc("TRN2", target_bir_lowering=False, debug=False)
    x_d = nc.declare_dram_parameter("x", [CH, POSC], F32, isOutput=False)
    o_d = nc.declare_dram_parameter("o", [CH, POSC], F32, isOutput=True)
    pd = {}
    for name, shape, dt in _param_specs():
        pd[name] = (nc.declare_dram_parameter(name, shape, dt, isOutput=False), dt)

    Tanh = mybir.ActivationFunctionType.Tanh
    mult = mybir.AluOpType.mult
    add = mybir.AluOpType.add

    with tile.TileContext(nc) as tc, ExitStack() as ctx:
        singles = ctx.enter_context(tc.tile_pool(name="singles", bufs=1))
        work = ctx.enter_context(tc.tile_pool(name="work", bufs=1))
        psum = ctx.enter_context(tc.tile_pool(name="psum", bufs=1, space="PSUM"))

        w = {}
        for name, (d, dt) in pd.items():
            t = singles.tile(list(d.shape), dt, tag=name, name=name)
            nc.sync.dma_start(out=t[:], in_=d[:])
            w[name] = t

        # resident staging tiles (allocated once; rewritten every repeat)
        xe = {P: singles.tile([R, HF], BF16, tag=f"xe{P}", name=f"xe{P}")
              for P in range(NPAT)}
        xet = singles.tile([TR, F], BF16, tag="xet", name="xet")
        osb = [singles.tile([R, OF], F32, tag=f"osb{g}", name=f"osb{g}")
               for g in range(2)]
        osbt = singles.tile([TU, F], F32, tag="osbt", name="osbt")

        xa = x_d[:]
        oa = o_d[:]

        def emit_rep(P):
            # x rows of pattern P replicated 3x j-major, f32 -> bf16 cast,
            # full-row 32 KB source descriptors; one DMA per j copy (a single
            # DMA with a zero-stride middle dim silently drops the replication)
            src = bass.AP(tensor=xa.tensor, offset=xa.offset + CHP * P * POSC,
                          ap=[[POSC, CHP], [1, POSC]])
            for j in range(3):
                nc.gpsimd.dma_start(out=xe[P][CHP * j:CHP * j + CHP, 0:HF],
                                    in_=src)

        def emit_tail_load():
            src = bass.AP(tensor=xa.tensor, offset=xa.offset + 252 * POSC,
                          ap=[[0, 3], [F, TU], [1, F]])
            nc.gpsimd.dma_start(out=xet[0:TR, 0:F], in_=src)

        def o_dst_ap(g, th):
            return bass.AP(tensor=oa.tensor,
                           offset=oa.offset + 126 * g * POSC + OF * th,
                           ap=[[POSC, 126], [1, OF]])

        def o_tail_ap():
            return bass.AP(tensor=oa.tensor, offset=oa.offset + 252 * POSC,
                           ap=[[POSC, TCH], [F, TB], [1, F]])

        def udims(u):
            """(rows, xrows, suffix) for unit u."""
            if u == NU:
                return TR, TU, "T"
            return R, CHP, str(u % NPAT)

        def xe_slice(u, lo, hi, cs0, cs1):
            """AP of unit u's x rows [lo,hi) cols [cs0,cs1) of F-chunk."""
            if u == NU:
                return xet[lo:hi, cs0:cs1]
            t, P = divmod(u, NPAT)
            base = t * F
            return xe[P][lo:hi, base + cs0:base + cs1]

        loop_cm = tc.For_i(0, repeat, 1) if repeat > 1 else nullcontext()

        def bootstrap():
            for P in range(NPAT):
                emit_rep(P)
            emit_tail_load()

        if do_dma:
            bootstrap()
        else:
            for t in list(xe.values()) + [xet]:
                nc.gpsimd.memset(t[:], 0.25)
            nc.gpsimd.memset(osb[0][:], 0.25)
            nc.gpsimd.memset(osb[1][:], 0.25)
            nc.gpsimd.memset(osbt[:], 0.25)
        if not do_compute:
            nc.gpsimd.memset(osb[0][:], 0.25)
            nc.gpsimd.memset(osb[1][:], 0.25)
            nc.gpsimd.memset(osbt[:], 0.25)

        with loop_cm:
            stB, stC, stD, stE, stG = {}, {}, {}, {}, {}
            packs, osbs = {}, {}
            SS = [slice(s * MMN, (s + 1) * MMN) for s in range(F // MMN)]

            def emit_B(u):
                rows, xr, sx = udims(u)
                t0 = work.tile([R, F], F32R, tag="t0", name="t0", bufs=3)
                nc.scalar.activation(t0[0:rows, :], xe_slice(u, 0, rows, 0, F),
                                     Tanh, bias=w["c0v" + sx][:],
                                     scale=w["W0v" + sx][:])
                stB[u] = t0

            def emit_C(u):
                rows, xr, sx = udims(u)
                t0 = stB.pop(u)
                y1 = psum.tile([R, F], F32, tag="y1", name="y1", bufs=2)
                for ss in SS:
                    nc.tensor.matmul(y1[0:rows, ss], w["l1t" + sx][:],
                                     t0[0:rows, ss], start=True, stop=False)
                for ss in SS:
                    nc.tensor.matmul(y1[0:rows, ss],
                                     w[("d1_" + sx) if u != NU else "d1T"][:],
                                     xe_slice(u, 0, xr, ss.start, ss.stop),
                                     start=False, stop=True)
                stC[u] = y1

            def emit_D(u):
                rows, _, sx = udims(u)
                y1 = stC[u]
                t1 = work.tile([R, F], F32R, tag="t1", name="t1", bufs=2)
                nc.scalar.activation(t1[0:rows, :], y1[0:rows, :], Tanh,
                                     bias=w["c1v" + sx][:])
                stD[u] = t1

            def emit_E(u):
                rows, _, sx = udims(u)
                y1 = stC.pop(u)
                t1 = stD.pop(u)
                z2 = work.tile([R, F], F32R, tag="z2", name="z2", bufs=2)
                nc.vector.scalar_tensor_tensor(
                    z2[0:rows, :], t1[0:rows, :], w["T1v" + sx][:],
                    y1[0:rows, :], op0=mult, op1=add)
                stE[u] = z2

            def emit_F(u):
                rows, _, sx = udims(u)
                z2 = stE[u]
                y2 = psum.tile([R, F], F32, tag="y2", name="y2", bufs=1)
                for ss in SS:
                    nc.tensor.matmul(y2[0:rows, ss],
                                     w[("l2_" + sx) if u != NU else "l2T"][:],
                                     z2[0:rows, ss], start=True, stop=True)
                stG[u] = y2

            def emit_G(u):
                rows, _, sx = udims(u)
                y2 = stG.pop(u)
                t2 = work.tile([R, F], F32R, tag="t2", name="t2", bufs=2)
                nc.scalar.activation(t2[0:rows, :], y2[0:rows, :], Tanh,
                                     bias=w["c2v" + sx][:])
                stD[("t2", u)] = t2

            def emit_H(u):
                rows, _, sx = udims(u)
                z2 = stE.pop(u)
                t2 = stD.pop(("t2", u))
                if u == NU:
                    pk = psum.tile([R, F], F32, tag="pack", name="pack", bufs=1)
                    for ss in SS:
                        nc.tensor.matmul(pk[0:TU, ss], w["l3zT"][:],
                                         z2[0:TR, ss], start=True, stop=False)
                    for ss in SS:
                        nc.tensor.matmul(pk[0:TU, ss], w["l3tT"][:],
                                         t2[0:TR, ss], start=False, stop=True)
                    nc.vector.tensor_scalar_add(osbt[0:TU, :], pk[0:TU, :],
                                                w["c3vT"][:])
                    if do_dma:
                        nc.sync.dma_start(out=o_tail_ap(),
                                          in_=osbt[0:TU, 0:F])
                    return
                t, P = divmod(u, NPAT)
                g = P // 3
                first, last = P % 3 == 0, P % 3 == 2
                if first:
                    packs[(t, g)] = psum.tile([R, F], F32, tag="pack",
                                              name="pack", bufs=1)
                pk = packs[(t, g)]
                for ss in SS:
                    nc.tensor.matmul(pk[:, ss], w["l3z" + sx][:], z2[:, ss],
                                     start=first, stop=False)
                for ss in SS:
                    nc.tensor.matmul(pk[:, ss], w["l3t" + sx][:], t2[:, ss],
                                     start=False, stop=last)
                if last:
                    pk = packs.pop((t, g))
                    th, ts = divmod(t, NCHUNK // 2)
                    nc.vector.tensor_scalar_add(
                        osb[g][:, ts * F:ts * F + F], pk[:],
                        w["c3vA" if g == 0 else "c3vB"][:])
                    if ts == NCHUNK // 2 - 1 and do_dma:
                        nc.sync.dma_start(out=o_dst_ap(g, th), in_=osb[g][:])

            if do_compute:
                # prologue
                for u in (0, 1, 2):
                    emit_B(u)
                emit_C(0)
                emit_D(0)
                emit_C(1)
                emit_D(1)
                emit_E(0)
                emit_F(0)
                emit_G(0)
                # steady state
                for i in range(NU + 1):
                    if i + 3 <= NU:
                        emit_B(i + 3)
                    if i + 2 <= NU:
                        emit_C(i + 2)
                        emit_D(i + 2)
                    if i + 1 <= NU:
                        emit_E(i + 1)
                        emit_F(i + 1)
                        emit_G(i + 1)
                    emit_H(i)
                    if do_dma and 40 <= i <= 45:
                        emit_rep(i - 40)   # xE reload for next repeat
                    if do_dma and i == 46:
                        emit_tail_load()
            else:
                for P in range(NPAT):
                    emit_rep(P)
                emit_tail_load()
                for g in range(2):
                    for th in range(2):
                        nc.sync.dma_start(out=o_dst_ap(g, th), in_=osb[g][:])
                nc.sync.dma_start(out=o_tail_ap(), in_=osbt[0:TU, 0:F])

    nc.finalize()
    return nc


def kernel(inputs, m0, m1, m2, m3, b0, b1, b2, b3, f0, f1, f2, stop_gradient):
    global LAST_RESULTS
    del stop_gradient  # False in setup_inputs; forward math identical anyway
    in_maps = make_in_maps(inputs, m0, m1, m2, m3, b0, b1, b2, b3, f0, f1, f2)

    nc = build_nc()
    res = run_bass_kernel_spmd(
        nc, in_maps, list(range(NCORES)),
        trace=bool(os.environ.get("BASS_TRACE")))
    LAST_RESULTS = res
    out = np.empty((CH, NPOS), dtype=np.float32)
    for g in range(NCORES):
        out[:, g * POSC:(g + 1) * POSC] = res.results[g]["o"]
    return out.reshape(CH, 1, NPOS)


def measure_exec_ns(in_maps, r1=8, r2=4104, n_wall=7):
    """Device-exec-time proxy: wall-clock delta between repeat=r2 and
    repeat=r1 kernels (upload/dispatch overheads cancel in the delta).
    min-of-many walls: the axon path adds multi-second host noise."""
    import time as _time
    walls = {}
    for rep in (r1, r2):
        nc = build_nc(repeat=rep)
        best = None
        for it in range(n_wall):
            t0 = _time.perf_counter()
            run_bass_kernel_spmd(nc, in_maps, list(range(NCORES)))
            dt = _time.perf_counter() - t0
            if it > 0:  # first call pays compile
                best = dt if best is None else min(best, dt)
        walls[rep] = best
    return (walls[r2] - walls[r1]) / (r2 - r1) * 1e9, walls


def make_in_maps(inputs, m0, m1, m2, m3, b0, b1, b2, b3, f0, f1, f2):
    inputs = np.asarray(inputs, dtype=np.float32)
    params = _host_params(
        *(np.asarray(a) for a in (m0, m1, m2, m3, b0, b1, b2, b3, f0, f1, f2)))
    arrs = _device_arrays(params)
    x = inputs.reshape(CH, NPOS)
    in_maps = []
    for g in range(NCORES):
        im = {"x": np.ascontiguousarray(x[:, g * POSC:(g + 1) * POSC])}
        im.update(arrs)
        in_maps.append(im)
    return in_maps
